# revision 1
# baseline (speedup 1.0000x reference)
"""MOELinearDGLFractional Trainium2 kernel.

Data-parallel over systems: 8 cores x 64 systems (512 rows each).
Per system s (rows r = 512*s + 4*q + j, q=partition, j=0..3):
  - DMA x tile [128, 1024] (4 rows/partition, contiguous 4KB/partition)
  - PE transpose 8x [128,128] blocks -> xT [i', rows]
  - fused matmul out[q, j*256+(moe|reg)] = sum_h xT_h.T @ V[:, :, h*64+s]
    where V = wmix3x [128(i'), 256(o: 0-127 moe, 128-255 reg), 128(h*64+b)]
  - DVE evac psum + bias -> out_sb, DMA out.
wmix (per-system mixed expert weights) produced on-PE in a prologue:
  one K=128-packed matmul per 8 (o,h)-chunks against block-diagonal coeff.
Regular Linear weights replicated into V's o=128..255 columns by GPSIMD.
"""

import sys

sys.path.insert(0, "/opt/trn_rl_repo")

import numpy as np

N_TOTAL = 262144
B = 512
E = 16
I_DIM = 256
O_MOE = 128
O_REG = 128
NCORES = 8
L = 512  # rows per system

import os

USE_F32R = os.environ.get("MOE_F32R", "0") == "1"  # float32r matmuls (1cyc/row @ N>=256, FP22)


def _mmdt(ap):
    import concourse.mybir as mybir

    return ap.bitcast(mybir.dt.float32r) if USE_F32R else ap


def build_program(n_sys):
    import concourse.bass as bass
    import concourse.mybir as mybir

    f32 = mybir.dt.float32
    rows = n_sys * L
    hb = 2 * n_sys  # (h, b) combined dim of V
    nldw = 32  # number of K=128-packed ldw groups (256 chunks / 8)
    pw_n = 8 * n_sys  # prod psum free size per group (8 chunks x n_sys)

    nc = bass.Bass()
    x = nc.declare_dram_parameter("x", [rows, I_DIM], f32, isOutput=False)
    wsb_d = nc.declare_dram_parameter("wsb", [128, 4096], f32, isOutput=False)
    c8t_d = nc.declare_dram_parameter("c8t", [128, pw_n], f32, isOutput=False)
    linwt_d = nc.declare_dram_parameter("linwt", [128, 256], f32, isOutput=False)
    bias_d = nc.declare_dram_parameter("bias2", [128, 512], f32, isOutput=False)
    ident_d = nc.declare_dram_parameter("ident", [128, 128], f32, isOutput=False)
    out = nc.declare_dram_parameter("out", [rows, 256], f32, isOutput=True)

    xv = x.rearrange("(s q j) m -> s q (j m)", q=128, j=4)
    ov = out.rearrange("(s q j) m -> s q (j m)", q=128, j=4)

    from contextlib import ExitStack

    with ExitStack() as ctx:
        en = ctx.enter_context
        wsb = en(nc.sbuf_tensor("wsb_sb", [128, 4096], f32))
        c8t = en(nc.sbuf_tensor("c8t_sb", [128, pw_n], f32))
        linwt = en(nc.sbuf_tensor("linwt_sb", [128, 256], f32))
        bias2 = en(nc.sbuf_tensor("bias2_sb", [128, 512], f32))
        ident = en(nc.sbuf_tensor("ident_sb", [128, 128], f32))
        # V: [i', o(256: moe|reg), hb]
        v3 = en(nc.sbuf_tensor("v3_sb", [128, 256, hb], f32))
        x_sb = [en(nc.sbuf_tensor(f"x_sb{i}", [128, 1024], f32)) for i in range(2)]
        xt_sb = [en(nc.sbuf_tensor(f"xt_sb{i}", [128, 1024], f32)) for i in range(2)]
        o_sb = [en(nc.sbuf_tensor(f"o_sb{i}", [128, 1024], f32)) for i in range(2)]
        # PSUM: 8 banks exactly
        xtp = [
            [en(nc.psum_tensor(f"xtp{i}{k}", [128, 512], f32)) for k in range(2)]
            for i in range(2)
        ]
        outp = [
            [en(nc.psum_tensor(f"outp{i}{k}", [128, 512], f32)) for k in range(2)]
            for i in range(2)
        ]

        # flat view of V's moe region for prologue evac
        v_moe_flat = v3.rearrange("p a b -> p (a b)")  # [128, 256*hb]

        sem_names = [
            "cst", "xin0", "xin1", "xp", "xt", "mm", "dve", "dout0", "dout1",
            "pw", "pwe",
        ]
        sems = {n: en(nc.semaphore(n)) for n in sem_names}
        cst_s, xp_s, xt_s, mm_s, dve_s, pw_s, pwe_s = (
            sems[n] for n in ["cst", "xp", "xt", "mm", "dve", "pw", "pwe"]
        )
        xin = [sems["xin0"], sems["xin1"]]
        dout = [sems["dout0"], sems["dout1"]]

        block = en(nc.Block())

        @block.sync
        def _(sync):
            sync.dma_start(out=wsb[:], in_=wsb_d[:]).then_inc(cst_s, 16)
            sync.dma_start(out=c8t[:], in_=c8t_d[:]).then_inc(cst_s, 16)
            sync.dma_start(out=linwt[:], in_=linwt_d[:]).then_inc(cst_s, 16)
            sync.dma_start(out=bias2[:], in_=bias_d[:]).then_inc(cst_s, 16)
            sync.dma_start(out=ident[:], in_=ident_d[:]).then_inc(cst_s, 16)
            for s in range(n_sys):
                if s >= 2:
                    sync.wait_ge(xp_s, s - 1)
                sync.dma_start(out=x_sb[s % 2][:], in_=xv[s]).then_inc(xin[s % 2], 16)

        @block.tensor
        def _(tensor):
            # ---- prologue: produce V moe columns (mixed expert weights) ----
            tensor.wait_ge(cst_s, 80)  # all const DMAs
            for g in range(nldw):
                if g >= 2:
                    tensor.wait_ge(pwe_s, g - 1)
                pp = outp[g % 2][0]
                for v in range(8):
                    inst = nc.tensor.matmul(
                        pp[:, v * n_sys : (v + 1) * n_sys],
                        wsb[:, g * 128 : (g + 1) * 128],
                        c8t[:, v * n_sys : (v + 1) * n_sys],
                        start=True,
                        stop=True,
                    )
                inst.then_inc(pw_s, 1)

            # ---- main loop ----
            def mms(s):
                # fused matmuls for system s (xt_sb already evacuated)
                tensor.wait_ge(xt_s, s + 1)
                if s == 0:
                    tensor.wait_ge(pwe_s, nldw)
                if s >= 2:
                    tensor.wait_ge(dve_s, s - 1)
                buf = s % 2
                for j in range(4):
                    pp = outp[buf][j // 2]
                    for h in range(2):
                        inst = nc.tensor.matmul(
                            pp[:, (j % 2) * 256 : (j % 2) * 256 + 256],
                            _mmdt(xt_sb[buf][:, (2 * j + h) * 128 : (2 * j + h + 1) * 128]),
                            _mmdt(v3[:, :, bass.ds(h * n_sys + s, 1)]),
                            start=(h == 0),
                            stop=(h == 1),
                        )
                inst.then_inc(mm_s, 1)

            for s in range(n_sys):
                # transposes for system s
                tensor.wait_ge(xin[s % 2], 16 * (s // 2 + 1))
                if s >= 2:
                    tensor.wait_ge(xt_s, s - 1)
                buf = s % 2
                for j in range(4):
                    for h in range(2):
                        k = 2 * j + h
                        inst = nc.tensor.transpose(
                            xtp[buf][k // 4][:, (k % 4) * 128 : (k % 4) * 128 + 128],
                            x_sb[buf][:, j * 256 + h * 128 : j * 256 + h * 128 + 128],
                            ident[:],
                        )
                inst.then_inc(xp_s, 1)
                if s >= 1:
                    mms(s - 1)
            mms(n_sys - 1)

        @block.scalar
        def _(scalar):
            # prologue: replicate linwt into V reg columns (overlaps PE prod)
            scalar.wait_ge(cst_s, 80)
            for h in range(2):
                for b in range(n_sys):
                    nc.scalar.copy(
                        out=_mmdt(v3[:, 128:256, bass.ds(h * n_sys + b, 1)]),
                        in_=linwt[:, h * 128 : (h + 1) * 128],
                    )
            # prologue evac: prod psum -> V moe region
            for g in range(nldw):
                scalar.wait_ge(pw_s, g + 1)
                nc.scalar.copy(
                    out=_mmdt(v_moe_flat[:, g * 8 * n_sys : (g + 1) * 8 * n_sys]),
                    in_=outp[g % 2][0][:, : 8 * n_sys],
                ).then_inc(pwe_s, 1)
            # main: xt psum -> xt_sb
            for s in range(n_sys):
                scalar.wait_ge(xp_s, s + 1)
                if s >= 2:
                    scalar.wait_ge(mm_s, s - 1)
                buf = s % 2
                nc.scalar.copy(out=_mmdt(xt_sb[buf][:, 0:512]), in_=xtp[buf][0][:])
                nc.scalar.copy(
                    out=_mmdt(xt_sb[buf][:, 512:1024]), in_=xtp[buf][1][:]
                ).then_inc(xt_s, 1)

        @block.vector
        def _(vector):
            for s in range(n_sys):
                vector.wait_ge(mm_s, s + 1)
                if s >= 2:
                    vector.wait_ge(dout[s % 2], 16 * (s // 2))
                buf = s % 2
                nc.vector.tensor_add(o_sb[buf][:, 0:512], outp[buf][0][:], bias2[:])
                nc.vector.tensor_add(
                    o_sb[buf][:, 512:1024], outp[buf][1][:], bias2[:]
                ).then_inc(dve_s, 1)

        @block.gpsimd
        def _(gpsimd):
            # main: out DMA
            for s in range(n_sys):
                gpsimd.wait_ge(dve_s, s + 1)
                gpsimd.dma_start(out=ov[s], in_=o_sb[s % 2][:]).then_inc(
                    dout[s % 2], 16
                )
            gpsimd.wait_ge(dout[0], 16 * ((n_sys + 1) // 2))
            gpsimd.wait_ge(dout[1], 16 * (n_sys // 2))

    return nc


def _host_inputs(x, coeff, moe_weights, moe_bias, lin_weight, lin_bias, n_sys, core):
    """Build per-core in_map."""
    # wsb: [16v+e, 128g+i'] = W[e, o(c), 128h(c)+i'], c=8g+v, (o,h)=divmod(c,2)
    Wr = np.asarray(moe_weights).reshape(E, 128, 2, 128)  # e,o,h,i'
    ch = Wr.transpose(1, 2, 0, 3).reshape(256, E, 128)  # c,e,i'
    wsb = np.ascontiguousarray(
        ch.reshape(32, 8, E, 128).transpose(1, 2, 0, 3).reshape(128, 4096)
    )
    b0 = core * n_sys
    cT = np.asarray(coeff)[b0 : b0 + n_sys].T.astype(np.float32)  # [E, n_sys]
    c8t = np.zeros((128, 8 * n_sys), np.float32)
    for v in range(8):
        c8t[16 * v : 16 * v + E, v * n_sys : (v + 1) * n_sys] = cT
    lw = np.asarray(lin_weight)  # [128, 256]
    linwt = np.ascontiguousarray(lw.reshape(128, 2, 128).transpose(2, 1, 0).reshape(128, 256))
    bias_cat = np.concatenate([np.asarray(moe_bias), np.asarray(lin_bias)]).astype(
        np.float32
    )
    bias2 = np.tile(bias_cat, (128, 2))
    ident = np.eye(128, dtype=np.float32)
    xs = np.ascontiguousarray(np.asarray(x)[core * n_sys * L : (core + 1) * n_sys * L])
    return {
        "x": xs,
        "wsb": wsb,
        "c8t": c8t,
        "linwt": linwt,
        "bias2": bias2,
        "ident": ident,
    }


_CACHE = {}


def kernel(
    x,
    expert_mixing_coefficients,
    routing_idxs,
    moe_weights,
    moe_bias,
    lin_weight,
    lin_bias,
    trace=False,
):
    from concourse.bass_utils import run_bass_kernel_spmd

    n_sys = B // NCORES
    if "nc" not in _CACHE:
        _CACHE["nc"] = build_program(n_sys)
    nc = _CACHE["nc"]
    in_maps = [
        _host_inputs(
            x, expert_mixing_coefficients, moe_weights, moe_bias, lin_weight, lin_bias,
            n_sys, c,
        )
        for c in range(NCORES)
    ]
    res = run_bass_kernel_spmd(nc, in_maps, list(range(NCORES)), trace=trace)
    outs = [res.results[c]["out"] for c in range(NCORES)]
    full = np.concatenate(outs, axis=0)
    if trace:
        return full, res
    return full



# revision 52
# speedup vs baseline: 331.6676x; 331.6676x over previous
"""MOELinearDGLFractional Trainium2 kernel.

Data-parallel over systems: 8 cores x 64 systems (512 rows each).

Per-core pipeline (s = system index, rows r = 512*s + 4*q + j; DMAs move
two systems = 1MB per transfer):
  - sync (HWDGE) DMA x pair-tile [128, 2048] fp32 -> x_sb
  - PE transpose 8x [128,128] fp32 blocks -> xtp psum
  - ACT evac xtp fp32 -> xt_sb bf16 [i', rows] (cast folded into evac)
  - PE matmuls (bf16, N=128): one accumulation group per psum bank,
    moe rhs = V[:, h*64+s, :] (contiguous), reg rhs = shared linwt tile
  - DVE evac psum + bias -> o_sb fp32, gpsimd (SWDGE) DMA out 1MB pairs.

Prologue: V = per-system mixed expert weights [i', hb, o] bf16, computed
on-PE as 32 merged bf16 matmuls (K=128-packed block-diagonal coeffs,
N=512, 4 psum banks deep) scatter-evacuated to bf16 by DVE.
"""

import sys

sys.path.insert(0, "/opt/trn_rl_repo")

import numpy as np
import ml_dtypes

N_TOTAL = 262144
B = 512
E = 16
I_DIM = 256
O_MOE = 128
O_REG = 128
NCORES = 8
L = 512  # rows per system

NB_X = 6  # x_sb pair buffers (1MB each)
NB_O = 3  # o_sb pair buffers (1MB each)


def build_program(n_sys):
    import concourse.bass as bass
    import concourse.mybir as mybir

    f32 = mybir.dt.float32
    bf16 = mybir.dt.bfloat16
    rows = n_sys * L
    hb = 2 * n_sys  # (h, b) combined dim of V
    nldw = 32  # prologue ldw groups (256 chunks / 8)
    pw_n = 8 * n_sys  # prologue psum free size per group
    npair = n_sys // 2

    nc = bass.Bass()
    x = nc.declare_dram_parameter("x", [rows, I_DIM], f32, isOutput=False)
    wsb_d = nc.declare_dram_parameter("wsb", [128, 4096], bf16, isOutput=False)
    c8t_d = nc.declare_dram_parameter("c8t", [128, pw_n], bf16, isOutput=False)
    linwt_d = nc.declare_dram_parameter("linwt", [128, 256], bf16, isOutput=False)
    bias_d = nc.declare_dram_parameter("bias2", [128, 512], f32, isOutput=False)
    ident_d = nc.declare_dram_parameter("ident", [128, 128], f32, isOutput=False)
    out = nc.declare_dram_parameter("out", [rows, 256], f32, isOutput=True)

    xv = x.rearrange("(t s2 q j) m -> t q s2 (j m)", s2=2, q=128, j=4)
    ov = out.rearrange("(t s2 q j) m -> t q s2 (j m)", s2=2, q=128, j=4)
    ov1 = out.rearrange("(s q j) m -> s q (j m)", q=128, j=4)  # 512KB view

    from contextlib import ExitStack

    with ExitStack() as ctx:
        en = ctx.enter_context
        wsb = en(nc.sbuf_tensor("wsb_sb", [128, 4096], bf16))
        c8t = en(nc.sbuf_tensor("c8t_sb", [128, pw_n], bf16))
        linwt = en(nc.sbuf_tensor("linwt_sb", [128, 256], bf16))
        bias2 = en(nc.sbuf_tensor("bias2_sb", [128, 512], f32))
        ident = en(nc.sbuf_tensor("ident_sb", [128, 128], f32))
        # V: [i', hb, o(128 moe)] bf16, o innermost (contiguous rhs)
        v3 = en(nc.sbuf_tensor("v3_sb", [128, hb, 128], bf16))
        x_sb = [en(nc.sbuf_tensor(f"x_sb{i}", [128, 2048], f32)) for i in range(NB_X)]
        xt_sb = [en(nc.sbuf_tensor(f"xt_sb{i}", [128, 1024], bf16)) for i in range(2)]
        o_sb = [en(nc.sbuf_tensor(f"o_sb{i}", [128, 2048], f32)) for i in range(NB_O)]
        # PSUM: 8 banks exactly
        xtp = [
            [en(nc.psum_tensor(f"xtp{i}{k}", [128, 512], f32)) for k in range(2)]
            for i in range(2)
        ]
        outp = [
            [en(nc.psum_tensor(f"outp{i}{k}", [128, 512], f32)) for k in range(2)]
            for i in range(2)
        ]

        sem_names = (
            ["cst", "wrm", "xp", "xt", "mm", "dve", "pw", "pweA", "pweB"]
            + [f"xin{i}" for i in range(NB_X)]
            + [f"dout{i}" for i in range(NB_O)]
        )
        sems = {n: en(nc.semaphore(n)) for n in sem_names}
        cst_s, xp_s, xt_s, mm_s, dve_s, pw_s, pweA_s, pweB_s = (
            sems[n] for n in ["cst", "xp", "xt", "mm", "dve", "pw", "pweA", "pweB"]
        )

        def wait_pwe(eng, g):
            # prologue evac of group g done (even g on DVE, odd on ACT)
            if g % 2 == 0:
                eng.wait_ge(pweA_s, g // 2 + 1)
            else:
                eng.wait_ge(pweB_s, (g + 1) // 2)
        xin = [sems[f"xin{i}"] for i in range(NB_X)]
        dout = [sems[f"dout{i}"] for i in range(NB_O)]

        block = en(nc.Block())

        @block.sync
        def _(sync):
            sync.dma_start(out=ident[:], in_=ident_d[:]).then_inc(sems["wrm"], 16)
            sync.dma_start(out=wsb[:], in_=wsb_d[:]).then_inc(cst_s, 16)
            sync.dma_start(out=c8t[:], in_=c8t_d[:]).then_inc(cst_s, 16)
            sync.dma_start(out=linwt[:], in_=linwt_d[:]).then_inc(cst_s, 16)
            sync.dma_start(out=bias2[:], in_=bias_d[:]).then_inc(cst_s, 16)
            # x in-DMA: 1MB per transfer (two systems)
            for t in range(npair):
                if t >= NB_X:
                    sync.wait_ge(xp_s, 2 * t - 2 * NB_X + 2)
                sync.dma_start(out=x_sb[t % NB_X][:], in_=xv[t]).then_inc(
                    xin[t % NB_X], 16
                )

        @block.gpsimd
        def _(gpsimd):
            # out DMA: 1MB per transfer (two systems); first pair split into
            # 512KB halves so the out stream starts one bias-evac earlier
            gpsimd.wait_ge(dve_s, 1)
            gpsimd.dma_start(out=ov1[0], in_=o_sb[0][:, 0:1024]).then_inc(dout[0], 16)
            gpsimd.wait_ge(dve_s, 2)
            gpsimd.dma_start(out=ov1[1], in_=o_sb[0][:, 1024:2048]).then_inc(
                dout[0], 16
            )
            for t in range(1, npair):
                gpsimd.wait_ge(dve_s, 2 * t + 2)
                gpsimd.dma_start(out=ov[t], in_=o_sb[t % NB_O][:]).then_inc(
                    dout[t % NB_O], 16
                )
            for b in range(NB_O):
                cnt = len([t for t in range(npair) if t % NB_O == b]) + (b == 0)
                gpsimd.wait_ge(dout[b], 16 * cnt)

        @block.tensor
        def _(tensor):
            def transposes(s):
                # PE-mode transposes straight off the fp32 x tile
                tensor.wait_ge(xin[(s // 2) % NB_X], 16 * (s // (2 * NB_X) + 1))
                if s >= 2:
                    tensor.wait_ge(xt_s, s - 1)
                buf = s % 2
                half = (s % 2) * 1024
                for j in range(4):
                    for h in range(2):
                        k = 2 * j + h
                        inst = nc.tensor.transpose(
                            xtp[buf][k // 4][:, (k % 4) * 128 : (k % 4) * 128 + 128],
                            x_sb[(s // 2) % NB_X][
                                :,
                                half + j * 256 + h * 128 : half + j * 256 + h * 128 + 128,
                            ],
                            ident[:],
                        )
                inst.then_inc(xp_s, 1)

            # HAM warm-up: ~3.4us of dummy fp32 matmuls on the identity while
            # the remaining consts stream in, so the prologue starts at the
            # un-throttled 2.4 GHz clock (results overwritten by start=True)
            tensor.wait_ge(sems["wrm"], 16)
            for _ in range(8):
                nc.tensor.matmul(
                    outp[0][0][:, 0:128], ident[:], ident[:], start=True, stop=True
                )

            # ---- prologue: V moe columns (mixed expert weights), bf16 ----
            tensor.wait_ge(cst_s, 64)
            for g in range(nldw):
                if g >= 4:
                    wait_pwe(tensor, g - 4)
                inst = nc.tensor.matmul(
                    outp[g % 2][(g // 2) % 2][:, 0:pw_n],
                    wsb[:, g * 128 : (g + 1) * 128],
                    c8t[:, 0:pw_n],
                    start=True,
                    stop=True,
                )
                inst.then_inc(pw_s, 1)
                if g == 19:
                    # first two systems' transposes: x pair 0 has landed by
                    # now; lets ACT's mid-prologue xt evacs proceed so
                    # mms(0) can launch right when the prologue completes
                    transposes(0)
                    transposes(1)

            # ---- main loop ----
            def mms(s):
                tensor.wait_ge(xt_s, s + 1)
                if s <= 1:
                    tensor.wait_ge(pweA_s, nldw // 2)
                    tensor.wait_ge(pweB_s, nldw // 2)
                if s >= 2:
                    tensor.wait_ge(dve_s, s - 1)
                buf = s % 2
                # one accumulation group per psum bank: start zeroes the
                # whole 2KB region, per-element has_written bits handle
                # first-touch-overwrite vs accumulate within the group
                for j in range(4):
                    pp = outp[buf][j // 2]
                    first = j % 2 == 0
                    last = j % 2 == 1
                    for h in range(2):
                        lhsT = xt_sb[buf][
                            :, (2 * j + h) * 128 : (2 * j + h + 1) * 128
                        ]
                        nc.tensor.matmul(
                            pp[:, (j % 2) * 256 : (j % 2) * 256 + 128],
                            lhsT,
                            v3[:, bass.ds(h * n_sys + s, 1), :],
                            start=(first and h == 0),
                            stop=False,
                        )
                        inst = nc.tensor.matmul(
                            pp[:, (j % 2) * 256 + 128 : (j % 2) * 256 + 256],
                            lhsT,
                            linwt[:, h * 128 : (h + 1) * 128],
                            start=False,
                            stop=(last and h == 1),
                        )
                inst.then_inc(mm_s, 1)

            for s in range(2, n_sys):
                transposes(s)
                mms(s - 2)
            mms(n_sys - 2)
            mms(n_sys - 1)

        def prologue_evac(eng, g, sem):
            eng.wait_ge(pw_s, g + 1)
            h = g // 16
            o0 = 8 * (g % 16)
            src = outp[g % 2][(g // 2) % 2][:, 0:pw_n].rearrange(
                "p (v b) -> p b v", v=8
            )
            dst = v3[:, h * n_sys : (h + 1) * n_sys, o0 : o0 + 8]
            if sem is pweA_s:
                nc.vector.tensor_copy(dst, src).then_inc(sem, 1)
            else:
                nc.scalar.copy(out=dst, in_=src).then_inc(sem, 1)

        @block.scalar
        def _(scalar):
            def evac_xt(s):
                scalar.wait_ge(xp_s, s + 1)
                if s >= 2:
                    scalar.wait_ge(mm_s, s - 1)
                buf = s % 2
                nc.scalar.copy(out=xt_sb[buf][:, 0:512], in_=xtp[buf][0][:])
                nc.scalar.copy(
                    out=xt_sb[buf][:, 512:1024], in_=xtp[buf][1][:]
                ).then_inc(xt_s, 1)

            # prologue evac: odd groups (even on DVE); slot the first two xt
            # evacs mid-sequence so mms(0) can launch right at pwe==32
            for g in range(1, nldw, 2):
                prologue_evac(scalar, g, pweB_s)
                if g == 15:
                    evac_xt(0)
                    evac_xt(1)
            # xt evac: psum fp32 -> xt_sb bf16 (cast folded into evac)
            for s in range(2, n_sys):
                evac_xt(s)

        @block.vector
        def _(vector):
            # prologue evac: psum [p, (v b)] -> V moe region [p, b(hb), o]
            # group g covers chunks c = 8g+v, all same h: o = 8*(g%16)+v
            # even groups here, odd groups on ACT
            for g in range(0, nldw, 2):
                prologue_evac(vector, g, pweA_s)
            # main: bias add evac into 1MB pair buffers (buffer 0's first use
            # produced two half-transfer incs, hence the +16 offset)
            for s in range(n_sys):
                vector.wait_ge(mm_s, s + 1)
                t = s // 2
                if t >= NB_O and s % 2 == 0:
                    vector.wait_ge(
                        dout[t % NB_O], 16 * (t // NB_O) + (16 if t % NB_O == 0 else 0)
                    )
                buf = s % 2
                half = (s % 2) * 1024
                nc.vector.tensor_add(
                    o_sb[t % NB_O][:, half : half + 512], outp[buf][0][:], bias2[:]
                )
                nc.vector.tensor_add(
                    o_sb[t % NB_O][:, half + 512 : half + 1024],
                    outp[buf][1][:],
                    bias2[:],
                ).then_inc(dve_s, 1)

    return nc


def _host_inputs(x, coeff, moe_weights, moe_bias, lin_weight, lin_bias, n_sys, core):
    """Build per-core in_map."""
    # wsb: [16v+e, 128g+i'] = W[e, o(c), 128h(c)+i'], c=8g+v, c = h*128+o
    Wr = np.asarray(moe_weights).reshape(E, 128, 2, 128)  # e,o,h,i'
    ch = Wr.transpose(2, 1, 0, 3).reshape(256, E, 128)  # c=(h,o),e,i'
    wsb = np.ascontiguousarray(
        ch.reshape(32, 8, E, 128).transpose(1, 2, 0, 3).reshape(128, 4096)
    ).astype(ml_dtypes.bfloat16)
    b0 = core * n_sys
    cT = np.asarray(coeff)[b0 : b0 + n_sys].T.astype(np.float32)  # [E, n_sys]
    c8t = np.zeros((128, 8 * n_sys), ml_dtypes.bfloat16)
    for v in range(8):
        c8t[16 * v : 16 * v + E, v * n_sys : (v + 1) * n_sys] = cT.astype(
            ml_dtypes.bfloat16
        )
    lw = np.asarray(lin_weight)  # [128, 256]
    linwt = np.ascontiguousarray(
        lw.reshape(128, 2, 128).transpose(2, 1, 0).reshape(128, 256)
    ).astype(ml_dtypes.bfloat16)
    bias_cat = np.concatenate([np.asarray(moe_bias), np.asarray(lin_bias)]).astype(
        np.float32
    )
    bias2 = np.tile(bias_cat, (128, 2))
    ident = np.eye(128, dtype=np.float32)
    xs = np.ascontiguousarray(np.asarray(x)[core * n_sys * L : (core + 1) * n_sys * L])
    return {
        "x": xs,
        "wsb": wsb,
        "c8t": c8t,
        "linwt": linwt,
        "bias2": bias2,
        "ident": ident,
    }


_CACHE = {}


def kernel(
    x,
    expert_mixing_coefficients,
    routing_idxs,
    moe_weights,
    moe_bias,
    lin_weight,
    lin_bias,
    trace=False,
    trace_cores=None,
):
    from concourse.bass_utils import run_bass_kernel_spmd

    n_sys = B // NCORES
    if "nc" not in _CACHE:
        _CACHE["nc"] = build_program(n_sys)
    nc = _CACHE["nc"]
    in_maps = [
        _host_inputs(
            x, expert_mixing_coefficients, moe_weights, moe_bias, lin_weight, lin_bias,
            n_sys, c,
        )
        for c in range(NCORES)
    ]
    res = run_bass_kernel_spmd(
        nc, in_maps, list(range(NCORES)), trace=trace, trace_cores=trace_cores
    )
    outs = [res.results[c]["out"] for c in range(NCORES)]
    full = np.concatenate(outs, axis=0)
    if trace:
        return full, res
    return full


# revision 55
# speedup vs baseline: 340.1832x; 1.0257x over previous
"""MOELinearDGLFractional Trainium2 kernel.

Data-parallel over systems: 8 cores x 64 systems (512 rows each).

Per-core pipeline (s = system index, rows r = 512*s + 4*q + j; DMAs move
two systems = 1MB per transfer):
  - sync (HWDGE) DMA x pair-tile [128, 2048] fp32 -> x_sb
  - PE transpose 8x [128,128] fp32 blocks -> xtp psum
  - ACT evac xtp fp32 -> xt_sb bf16 [i', rows] (cast folded into evac)
  - PE matmuls (bf16, N=128): one accumulation group per psum bank,
    moe rhs = V[:, h*64+s, :] (contiguous), reg rhs = shared linwt tile
  - DVE evac psum + bias -> o_sb fp32, gpsimd (SWDGE) DMA out 1MB pairs.

Prologue: V = per-system mixed expert weights [i', hb, o] bf16, computed
on-PE as 32 merged bf16 matmuls (K=128-packed block-diagonal coeffs,
N=512, 4 psum banks deep) scatter-evacuated to bf16 by DVE.
"""

import sys

sys.path.insert(0, "/opt/trn_rl_repo")

import numpy as np
import ml_dtypes

N_TOTAL = 262144
B = 512
E = 16
I_DIM = 256
O_MOE = 128
O_REG = 128
NCORES = 8
L = 512  # rows per system

NB_X = 6  # x_sb pair buffers (1MB each)
NB_O = 3  # o_sb pair buffers (1MB each)


def build_program(n_sys):
    import concourse.bass as bass
    import concourse.mybir as mybir

    f32 = mybir.dt.float32
    bf16 = mybir.dt.bfloat16
    rows = n_sys * L
    hb = 2 * n_sys  # (h, b) combined dim of V
    nldw = 32  # prologue ldw groups (256 chunks / 8)
    pw_n = 8 * n_sys  # prologue psum free size per group
    npair = n_sys // 2

    nc = bass.Bass()
    x = nc.declare_dram_parameter("x", [rows, I_DIM], f32, isOutput=False)
    wsb_d = nc.declare_dram_parameter("wsb", [128, 4096], bf16, isOutput=False)
    c8t_d = nc.declare_dram_parameter("c8t", [128, pw_n], bf16, isOutput=False)
    linwt_d = nc.declare_dram_parameter("linwt", [128, 256], bf16, isOutput=False)
    bias_d = nc.declare_dram_parameter("bias2", [128, 512], f32, isOutput=False)
    ident_d = nc.declare_dram_parameter("ident", [128, 128], f32, isOutput=False)
    out = nc.declare_dram_parameter("out", [rows, 256], f32, isOutput=True)

    xv = x.rearrange("(t s2 q j) m -> t q s2 (j m)", s2=2, q=128, j=4)
    ov = out.rearrange("(t s2 q j) m -> t q s2 (j m)", s2=2, q=128, j=4)
    ov1 = out.rearrange("(s q j) m -> s q (j m)", q=128, j=4)  # 512KB view

    from contextlib import ExitStack

    with ExitStack() as ctx:
        en = ctx.enter_context
        wsb = en(nc.sbuf_tensor("wsb_sb", [128, 4096], bf16))
        c8t = en(nc.sbuf_tensor("c8t_sb", [128, pw_n], bf16))
        linwt = en(nc.sbuf_tensor("linwt_sb", [128, 256], bf16))
        bias2 = en(nc.sbuf_tensor("bias2_sb", [128, 512], f32))
        ident = en(nc.sbuf_tensor("ident_sb", [128, 128], f32))
        # V: [i', hb, o(128 moe)] bf16, o innermost (contiguous rhs)
        v3 = en(nc.sbuf_tensor("v3_sb", [128, hb, 128], bf16))
        x_sb = [en(nc.sbuf_tensor(f"x_sb{i}", [128, 2048], f32)) for i in range(NB_X)]
        xt_sb = [en(nc.sbuf_tensor(f"xt_sb{i}", [128, 1024], bf16)) for i in range(2)]
        o_sb = [en(nc.sbuf_tensor(f"o_sb{i}", [128, 2048], f32)) for i in range(NB_O)]
        # PSUM: 8 banks exactly
        xtp = [
            [en(nc.psum_tensor(f"xtp{i}{k}", [128, 512], f32)) for k in range(2)]
            for i in range(2)
        ]
        outp = [
            [en(nc.psum_tensor(f"outp{i}{k}", [128, 512], f32)) for k in range(2)]
            for i in range(2)
        ]

        sem_names = (
            ["cst", "wrm", "xp", "xt", "mm", "dve", "pw", "pweA", "pweB"]
            + [f"xin{i}" for i in range(NB_X)]
            + [f"dout{i}" for i in range(NB_O)]
        )
        sems = {n: en(nc.semaphore(n)) for n in sem_names}
        cst_s, xp_s, xt_s, mm_s, dve_s, pw_s, pweA_s, pweB_s = (
            sems[n] for n in ["cst", "xp", "xt", "mm", "dve", "pw", "pweA", "pweB"]
        )

        def wait_pwe(eng, g):
            # prologue evac of group g done (even g on DVE, odd on ACT)
            if g % 2 == 0:
                eng.wait_ge(pweA_s, g // 2 + 1)
            else:
                eng.wait_ge(pweB_s, (g + 1) // 2)
        xin = [sems[f"xin{i}"] for i in range(NB_X)]
        dout = [sems[f"dout{i}"] for i in range(NB_O)]

        block = en(nc.Block())

        @block.sync
        def _(sync):
            sync.dma_start(out=ident[:], in_=ident_d[:]).then_inc(sems["wrm"], 16)
            sync.dma_start(out=wsb[:], in_=wsb_d[:]).then_inc(cst_s, 16)
            sync.dma_start(out=c8t[:], in_=c8t_d[:]).then_inc(cst_s, 16)
            sync.dma_start(out=linwt[:], in_=linwt_d[:]).then_inc(cst_s, 16)
            sync.dma_start(out=bias2[:], in_=bias_d[:]).then_inc(cst_s, 16)
            # x in-DMA: 1MB per transfer (two systems); the first NB_X
            # wait-free transfers are issued from the scalar HWDGE ring so
            # the x stream starts concurrently with the consts
            for t in range(NB_X, npair):
                sync.wait_ge(xp_s, 2 * t - 2 * NB_X + 2)
                sync.dma_start(out=x_sb[t % NB_X][:], in_=xv[t]).then_inc(
                    xin[t % NB_X], 16
                )

        @block.gpsimd
        def _(gpsimd):
            # out DMA: 1MB per transfer (two systems); first pair split into
            # 512KB halves so the out stream starts one bias-evac earlier
            gpsimd.wait_ge(dve_s, 1)
            gpsimd.dma_start(out=ov1[0], in_=o_sb[0][:, 0:1024]).then_inc(dout[0], 16)
            gpsimd.wait_ge(dve_s, 2)
            gpsimd.dma_start(out=ov1[1], in_=o_sb[0][:, 1024:2048]).then_inc(
                dout[0], 16
            )
            for t in range(1, npair - 1):
                gpsimd.wait_ge(dve_s, 2 * t + 2)
                gpsimd.dma_start(out=ov[t], in_=o_sb[t % NB_O][:]).then_inc(
                    dout[t % NB_O], 16
                )
            # last pair split into halves: the kernel's final DMA completion
            # then covers only 512KB, shortening the tail
            tl = npair - 1
            gpsimd.wait_ge(dve_s, 2 * tl + 1)
            gpsimd.dma_start(
                out=ov1[2 * tl], in_=o_sb[tl % NB_O][:, 0:1024]
            ).then_inc(dout[tl % NB_O], 16)
            gpsimd.wait_ge(dve_s, 2 * tl + 2)
            gpsimd.dma_start(
                out=ov1[2 * tl + 1], in_=o_sb[tl % NB_O][:, 1024:2048]
            ).then_inc(dout[tl % NB_O], 16)
            for b in range(NB_O):
                cnt = (
                    len([t for t in range(npair) if t % NB_O == b])
                    + (b == 0)
                    + (b == tl % NB_O)
                )
                gpsimd.wait_ge(dout[b], 16 * cnt)

        @block.tensor
        def _(tensor):
            def transposes(s):
                # PE-mode transposes straight off the fp32 x tile
                tensor.wait_ge(xin[(s // 2) % NB_X], 16 * (s // (2 * NB_X) + 1))
                if s >= 2:
                    tensor.wait_ge(xt_s, s - 1)
                buf = s % 2
                half = (s % 2) * 1024
                for j in range(4):
                    for h in range(2):
                        k = 2 * j + h
                        inst = nc.tensor.transpose(
                            xtp[buf][k // 4][:, (k % 4) * 128 : (k % 4) * 128 + 128],
                            x_sb[(s // 2) % NB_X][
                                :,
                                half + j * 256 + h * 128 : half + j * 256 + h * 128 + 128,
                            ],
                            ident[:],
                        )
                inst.then_inc(xp_s, 1)

            # HAM warm-up: ~3.4us of dummy fp32 matmuls on the identity while
            # the remaining consts stream in, so the prologue starts at the
            # un-throttled 2.4 GHz clock (results overwritten by start=True)
            tensor.wait_ge(sems["wrm"], 16)
            for _ in range(8):
                nc.tensor.matmul(
                    outp[0][0][:, 0:128], ident[:], ident[:], start=True, stop=True
                )

            # ---- prologue: V moe columns (mixed expert weights), bf16 ----
            tensor.wait_ge(cst_s, 64)
            for g in range(nldw):
                if g >= 4:
                    wait_pwe(tensor, g - 4)
                inst = nc.tensor.matmul(
                    outp[g % 2][(g // 2) % 2][:, 0:pw_n],
                    wsb[:, g * 128 : (g + 1) * 128],
                    c8t[:, 0:pw_n],
                    start=True,
                    stop=True,
                )
                inst.then_inc(pw_s, 1)
                if g == 19:
                    # first two systems' transposes: x pair 0 has landed by
                    # now; lets ACT's mid-prologue xt evacs proceed so
                    # mms(0) can launch right when the prologue completes
                    transposes(0)
                    transposes(1)

            # ---- main loop ----
            def mms(s):
                tensor.wait_ge(xt_s, s + 1)
                if s <= 1:
                    tensor.wait_ge(pweA_s, nldw // 2)
                    tensor.wait_ge(pweB_s, nldw // 2)
                if s >= 2:
                    tensor.wait_ge(dve_s, s - 1)
                buf = s % 2
                # one accumulation group per psum bank: start zeroes the
                # whole 2KB region, per-element has_written bits handle
                # first-touch-overwrite vs accumulate within the group
                for j in range(4):
                    pp = outp[buf][j // 2]
                    first = j % 2 == 0
                    last = j % 2 == 1
                    for h in range(2):
                        lhsT = xt_sb[buf][
                            :, (2 * j + h) * 128 : (2 * j + h + 1) * 128
                        ]
                        nc.tensor.matmul(
                            pp[:, (j % 2) * 256 : (j % 2) * 256 + 128],
                            lhsT,
                            v3[:, bass.ds(h * n_sys + s, 1), :],
                            start=(first and h == 0),
                            stop=False,
                        )
                        inst = nc.tensor.matmul(
                            pp[:, (j % 2) * 256 + 128 : (j % 2) * 256 + 256],
                            lhsT,
                            linwt[:, h * 128 : (h + 1) * 128],
                            start=False,
                            stop=(last and h == 1),
                        )
                inst.then_inc(mm_s, 1)

            for s in range(2, n_sys):
                transposes(s)
                mms(s - 2)
            mms(n_sys - 2)
            mms(n_sys - 1)

        def prologue_evac(eng, g, sem):
            eng.wait_ge(pw_s, g + 1)
            h = g // 16
            o0 = 8 * (g % 16)
            src = outp[g % 2][(g // 2) % 2][:, 0:pw_n].rearrange(
                "p (v b) -> p b v", v=8
            )
            dst = v3[:, h * n_sys : (h + 1) * n_sys, o0 : o0 + 8]
            if sem is pweA_s:
                nc.vector.tensor_copy(dst, src).then_inc(sem, 1)
            else:
                nc.scalar.copy(out=dst, in_=src).then_inc(sem, 1)

        @block.scalar
        def _(scalar):
            # first x transfers, wait-free, concurrent with consts on sync
            for t in range(min(NB_X, npair)):
                scalar.dma_start(out=x_sb[t][:], in_=xv[t]).then_inc(xin[t], 16)

            def evac_xt(s):
                scalar.wait_ge(xp_s, s + 1)
                if s >= 2:
                    scalar.wait_ge(mm_s, s - 1)
                buf = s % 2
                nc.scalar.copy(out=xt_sb[buf][:, 0:512], in_=xtp[buf][0][:])
                nc.scalar.copy(
                    out=xt_sb[buf][:, 512:1024], in_=xtp[buf][1][:]
                ).then_inc(xt_s, 1)

            # prologue evac: odd groups (even on DVE); slot the first two xt
            # evacs mid-sequence so mms(0) can launch right at pwe==32
            for g in range(1, nldw, 2):
                prologue_evac(scalar, g, pweB_s)
                if g == 15:
                    evac_xt(0)
                    evac_xt(1)
            # xt evac: psum fp32 -> xt_sb bf16 (cast folded into evac)
            for s in range(2, n_sys):
                evac_xt(s)

        @block.vector
        def _(vector):
            # prologue evac: psum [p, (v b)] -> V moe region [p, b(hb), o]
            # group g covers chunks c = 8g+v, all same h: o = 8*(g%16)+v
            # even groups here, odd groups on ACT
            for g in range(0, nldw, 2):
                prologue_evac(vector, g, pweA_s)
            # main: bias add evac into 1MB pair buffers (buffer 0's first use
            # produced two half-transfer incs, hence the +16 offset)
            for s in range(n_sys):
                vector.wait_ge(mm_s, s + 1)
                t = s // 2
                if t >= NB_O and s % 2 == 0:
                    vector.wait_ge(
                        dout[t % NB_O], 16 * (t // NB_O) + (16 if t % NB_O == 0 else 0)
                    )
                buf = s % 2
                half = (s % 2) * 1024
                nc.vector.tensor_add(
                    o_sb[t % NB_O][:, half : half + 512], outp[buf][0][:], bias2[:]
                )
                nc.vector.tensor_add(
                    o_sb[t % NB_O][:, half + 512 : half + 1024],
                    outp[buf][1][:],
                    bias2[:],
                ).then_inc(dve_s, 1)

    return nc


def _host_inputs(x, coeff, moe_weights, moe_bias, lin_weight, lin_bias, n_sys, core):
    """Build per-core in_map."""
    # wsb: [16v+e, 128g+i'] = W[e, o(c), 128h(c)+i'], c=8g+v, c = h*128+o
    Wr = np.asarray(moe_weights).reshape(E, 128, 2, 128)  # e,o,h,i'
    ch = Wr.transpose(2, 1, 0, 3).reshape(256, E, 128)  # c=(h,o),e,i'
    wsb = np.ascontiguousarray(
        ch.reshape(32, 8, E, 128).transpose(1, 2, 0, 3).reshape(128, 4096)
    ).astype(ml_dtypes.bfloat16)
    b0 = core * n_sys
    cT = np.asarray(coeff)[b0 : b0 + n_sys].T.astype(np.float32)  # [E, n_sys]
    c8t = np.zeros((128, 8 * n_sys), ml_dtypes.bfloat16)
    for v in range(8):
        c8t[16 * v : 16 * v + E, v * n_sys : (v + 1) * n_sys] = cT.astype(
            ml_dtypes.bfloat16
        )
    lw = np.asarray(lin_weight)  # [128, 256]
    linwt = np.ascontiguousarray(
        lw.reshape(128, 2, 128).transpose(2, 1, 0).reshape(128, 256)
    ).astype(ml_dtypes.bfloat16)
    bias_cat = np.concatenate([np.asarray(moe_bias), np.asarray(lin_bias)]).astype(
        np.float32
    )
    bias2 = np.tile(bias_cat, (128, 2))
    ident = np.eye(128, dtype=np.float32)
    xs = np.ascontiguousarray(np.asarray(x)[core * n_sys * L : (core + 1) * n_sys * L])
    return {
        "x": xs,
        "wsb": wsb,
        "c8t": c8t,
        "linwt": linwt,
        "bias2": bias2,
        "ident": ident,
    }


_CACHE = {}


def kernel(
    x,
    expert_mixing_coefficients,
    routing_idxs,
    moe_weights,
    moe_bias,
    lin_weight,
    lin_bias,
    trace=False,
    trace_cores=None,
):
    from concourse.bass_utils import run_bass_kernel_spmd

    n_sys = B // NCORES
    if "nc" not in _CACHE:
        _CACHE["nc"] = build_program(n_sys)
    nc = _CACHE["nc"]
    in_maps = [
        _host_inputs(
            x, expert_mixing_coefficients, moe_weights, moe_bias, lin_weight, lin_bias,
            n_sys, c,
        )
        for c in range(NCORES)
    ]
    res = run_bass_kernel_spmd(
        nc, in_maps, list(range(NCORES)), trace=trace, trace_cores=trace_cores
    )
    outs = [res.results[c]["out"] for c in range(NCORES)]
    full = np.concatenate(outs, axis=0)
    if trace:
        return full, res
    return full


# revision 62
# speedup vs baseline: 414.6902x; 1.2190x over previous
"""MOELinearDGLFractional Trainium2 kernel.

Data-parallel over systems: 8 cores x 64 systems (512 rows each).

Per-core pipeline (s = system index, rows r = 512*s + 4*q + j; DMAs move
two systems = 1MB per transfer):
  - sync (HWDGE) DMA x pair-tile [128, 2048] fp32 -> x_sb
  - PE transpose 8x [128,128] fp32 blocks -> xtp psum
  - ACT evac xtp fp32 -> xt_sb bf16 [i', rows] (cast folded into evac)
  - PE matmuls (bf16, N=128): one accumulation group per psum bank,
    moe rhs = V[:, h*64+s, :] (contiguous), reg rhs = shared linwt tile
  - DVE evac psum + bias -> o_sb fp32, gpsimd (SWDGE) DMA out 1MB pairs.

Prologue: V = per-system mixed expert weights [i', hb, o] bf16, computed
on-PE as 32 merged bf16 matmuls (K=128-packed block-diagonal coeffs,
N=512, 4 psum banks deep) scatter-evacuated to bf16 by DVE.
"""

import sys

sys.path.insert(0, "/opt/trn_rl_repo")

import numpy as np
import ml_dtypes

N_TOTAL = 262144
B = 512
E = 16
I_DIM = 256
O_MOE = 128
O_REG = 128
NCORES = 8
L = 512  # rows per system

NB_X = 6  # x_sb pair buffers (1MB each)
NB_O = 3  # o_sb pair buffers (1MB each)


def build_program(n_sys):
    import concourse.bass as bass
    import concourse.mybir as mybir

    f32 = mybir.dt.float32
    bf16 = mybir.dt.bfloat16
    rows = n_sys * L
    hb = 2 * n_sys  # (h, b) combined dim of V
    nldw = 32  # prologue ldw groups (256 chunks / 8)
    pw_n = 8 * n_sys  # prologue psum free size per group
    npair = n_sys // 2

    nc = bass.Bass()
    x = nc.declare_dram_parameter("x", [rows, I_DIM], bf16, isOutput=False)
    wsb_d = nc.declare_dram_parameter("wsb", [128, 4096], bf16, isOutput=False)
    c8t_d = nc.declare_dram_parameter("c8t", [128, pw_n], bf16, isOutput=False)
    linwt_d = nc.declare_dram_parameter("linwt", [128, 256], bf16, isOutput=False)
    bias_d = nc.declare_dram_parameter("bias2", [128, 512], f32, isOutput=False)
    ident_d = nc.declare_dram_parameter("ident", [128, 128], bf16, isOutput=False)
    out = nc.declare_dram_parameter("out", [rows, 256], f32, isOutput=True)

    xv = x.rearrange("(t s2 q j) m -> t q s2 (j m)", s2=2, q=128, j=4)
    ov = out.rearrange("(t s2 q j) m -> t q s2 (j m)", s2=2, q=128, j=4)
    ov1 = out.rearrange("(s q j) m -> s q (j m)", q=128, j=4)  # 512KB view

    from contextlib import ExitStack

    with ExitStack() as ctx:
        en = ctx.enter_context
        wsb = en(nc.sbuf_tensor("wsb_sb", [128, 4096], bf16))
        c8t = en(nc.sbuf_tensor("c8t_sb", [128, pw_n], bf16))
        linwt = en(nc.sbuf_tensor("linwt_sb", [128, 256], bf16))
        bias2 = en(nc.sbuf_tensor("bias2_sb", [128, 512], f32))
        ident = en(nc.sbuf_tensor("ident_sb", [128, 128], bf16))
        # V: [i', hb, o(128 moe)] bf16, o innermost (contiguous rhs)
        v3 = en(nc.sbuf_tensor("v3_sb", [128, hb, 128], bf16))
        x_sb = [en(nc.sbuf_tensor(f"x_sb{i}", [128, 2048], bf16)) for i in range(NB_X)]
        xt_sb = [en(nc.sbuf_tensor(f"xt_sb{i}", [128, 1024], bf16)) for i in range(2)]
        o_sb = [en(nc.sbuf_tensor(f"o_sb{i}", [128, 2048], f32)) for i in range(NB_O)]
        # PSUM: 8 banks exactly
        xtp = [
            [en(nc.psum_tensor(f"xtp{i}{k}", [128, 512], f32)) for k in range(2)]
            for i in range(2)
        ]
        outp = [
            [en(nc.psum_tensor(f"outp{i}{k}", [128, 512], f32)) for k in range(2)]
            for i in range(2)
        ]

        sem_names = (
            ["cst", "wrm", "xp", "xt", "mm", "dve", "pw", "pweA", "pweB"]
            + [f"xin{i}" for i in range(NB_X)]
            + [f"dout{i}" for i in range(NB_O)]
        )
        sems = {n: en(nc.semaphore(n)) for n in sem_names}
        cst_s, xp_s, xt_s, mm_s, dve_s, pw_s, pweA_s, pweB_s = (
            sems[n] for n in ["cst", "xp", "xt", "mm", "dve", "pw", "pweA", "pweB"]
        )

        def wait_pwe(eng, g):
            # prologue evac of group g done (even g on DVE, odd on ACT)
            if g % 2 == 0:
                eng.wait_ge(pweA_s, g // 2 + 1)
            else:
                eng.wait_ge(pweB_s, (g + 1) // 2)
        xin = [sems[f"xin{i}"] for i in range(NB_X)]
        dout = [sems[f"dout{i}"] for i in range(NB_O)]

        block = en(nc.Block())

        @block.sync
        def _(sync):
            sync.dma_start(out=ident[:], in_=ident_d[:]).then_inc(sems["wrm"], 16)
            sync.dma_start(out=wsb[:], in_=wsb_d[:]).then_inc(cst_s, 16)
            sync.dma_start(out=c8t[:], in_=c8t_d[:]).then_inc(cst_s, 16)
            sync.dma_start(out=linwt[:], in_=linwt_d[:]).then_inc(cst_s, 16)
            sync.dma_start(out=bias2[:], in_=bias_d[:]).then_inc(cst_s, 16)
            # x in-DMA: 1MB per transfer (two systems); the first NB_X
            # wait-free transfers are issued from the scalar HWDGE ring so
            # the x stream starts concurrently with the consts
            for t in range(NB_X, npair):
                sync.wait_ge(xp_s, 2 * t - 2 * NB_X + 2)
                sync.dma_start(out=x_sb[t % NB_X][:], in_=xv[t]).then_inc(
                    xin[t % NB_X], 16
                )

        @block.gpsimd
        def _(gpsimd):
            # out DMA: 1MB per transfer (two systems); first pair split into
            # 512KB halves so the out stream starts one bias-evac earlier
            gpsimd.wait_ge(dve_s, 1)
            gpsimd.dma_start(out=ov1[0], in_=o_sb[0][:, 0:1024]).then_inc(dout[0], 16)
            gpsimd.wait_ge(dve_s, 2)
            gpsimd.dma_start(out=ov1[1], in_=o_sb[0][:, 1024:2048]).then_inc(
                dout[0], 16
            )
            for t in range(1, npair - 1):
                gpsimd.wait_ge(dve_s, 2 * t + 2)
                gpsimd.dma_start(out=ov[t], in_=o_sb[t % NB_O][:]).then_inc(
                    dout[t % NB_O], 16
                )
            # last pair split into halves: the kernel's final DMA completion
            # then covers only 512KB, shortening the tail
            tl = npair - 1
            gpsimd.wait_ge(dve_s, 2 * tl + 1)
            gpsimd.dma_start(
                out=ov1[2 * tl], in_=o_sb[tl % NB_O][:, 0:1024]
            ).then_inc(dout[tl % NB_O], 16)
            gpsimd.wait_ge(dve_s, 2 * tl + 2)
            gpsimd.dma_start(
                out=ov1[2 * tl + 1], in_=o_sb[tl % NB_O][:, 1024:2048]
            ).then_inc(dout[tl % NB_O], 16)
            for b in range(NB_O):
                cnt = (
                    len([t for t in range(npair) if t % NB_O == b])
                    + (b == 0)
                    + (b == tl % NB_O)
                )
                gpsimd.wait_ge(dout[b], 16 * cnt)

        @block.tensor
        def _(tensor):
            def transposes(s):
                # transposes as regular bf16 matmuls vs identity (1 cyc/col,
                # and they count as PE-busy for the HAM clock-gate)
                tensor.wait_ge(xin[(s // 2) % NB_X], 16 * (s // (2 * NB_X) + 1))
                if s >= 2:
                    tensor.wait_ge(xt_s, s - 1)
                buf = s % 2
                half = (s % 2) * 1024
                for j in range(4):
                    for h in range(2):
                        k = 2 * j + h
                        inst = nc.tensor.matmul(
                            xtp[buf][k // 4][:, (k % 4) * 128 : (k % 4) * 128 + 128],
                            x_sb[(s // 2) % NB_X][
                                :,
                                half + j * 256 + h * 128 : half + j * 256 + h * 128 + 128,
                            ],
                            ident[:],
                            start=True,
                            stop=True,
                        )
                inst.then_inc(xp_s, 1)

            # HAM warm-up: ~3.4us of dummy fp32 matmuls on the identity while
            # the remaining consts stream in, so the prologue starts at the
            # un-throttled 2.4 GHz clock (results overwritten by start=True)
            tensor.wait_ge(sems["wrm"], 16)
            for _ in range(16):
                nc.tensor.matmul(
                    outp[0][0][:, 0:128], ident[:], ident[:], start=True, stop=True
                )

            # ---- prologue: V moe columns (mixed expert weights), bf16 ----
            tensor.wait_ge(cst_s, 64)
            for g in range(nldw):
                if g >= 4:
                    wait_pwe(tensor, g - 4)
                inst = nc.tensor.matmul(
                    outp[g % 2][(g // 2) % 2][:, 0:pw_n],
                    wsb[:, g * 128 : (g + 1) * 128],
                    c8t[:, 0:pw_n],
                    start=True,
                    stop=True,
                )
                inst.then_inc(pw_s, 1)
                if g == 19:
                    # first two systems' transposes: x pair 0 has landed by
                    # now; lets ACT's mid-prologue xt evacs proceed so
                    # mms(0) can launch right when the prologue completes
                    transposes(0)
                    transposes(1)

            # ---- main loop ----
            def mms(s):
                tensor.wait_ge(xt_s, s + 1)
                if s <= 1:
                    tensor.wait_ge(pweA_s, nldw // 2)
                    tensor.wait_ge(pweB_s, nldw // 2)
                if s >= 2:
                    tensor.wait_ge(dve_s, s - 1)
                buf = s % 2
                # one accumulation group per psum bank: start zeroes the
                # whole 2KB region, per-element has_written bits handle
                # first-touch-overwrite vs accumulate within the group
                for j in range(4):
                    pp = outp[buf][j // 2]
                    first = j % 2 == 0
                    last = j % 2 == 1
                    for h in range(2):
                        lhsT = xt_sb[buf][
                            :, (2 * j + h) * 128 : (2 * j + h + 1) * 128
                        ]
                        nc.tensor.matmul(
                            pp[:, (j % 2) * 256 : (j % 2) * 256 + 128],
                            lhsT,
                            v3[:, bass.ds(h * n_sys + s, 1), :],
                            start=(first and h == 0),
                            stop=False,
                        )
                        inst = nc.tensor.matmul(
                            pp[:, (j % 2) * 256 + 128 : (j % 2) * 256 + 256],
                            lhsT,
                            linwt[:, h * 128 : (h + 1) * 128],
                            start=False,
                            stop=(last and h == 1),
                        )
                inst.then_inc(mm_s, 1)

            for s in range(2, n_sys):
                transposes(s)
                mms(s - 2)
            mms(n_sys - 2)
            mms(n_sys - 1)

        def prologue_evac(eng, g, sem):
            eng.wait_ge(pw_s, g + 1)
            h = g // 16
            o0 = 8 * (g % 16)
            src = outp[g % 2][(g // 2) % 2][:, 0:pw_n].rearrange(
                "p (v b) -> p b v", v=8
            )
            dst = v3[:, h * n_sys : (h + 1) * n_sys, o0 : o0 + 8]
            if sem is pweA_s:
                nc.vector.tensor_copy(dst, src).then_inc(sem, 1)
            else:
                nc.scalar.copy(out=dst, in_=src).then_inc(sem, 1)

        @block.scalar
        def _(scalar):
            # first x transfers, wait-free, concurrent with consts on sync
            for t in range(min(NB_X, npair)):
                scalar.dma_start(out=x_sb[t][:], in_=xv[t]).then_inc(xin[t], 16)

            def evac_xt(s):
                scalar.wait_ge(xp_s, s + 1)
                if s >= 2:
                    scalar.wait_ge(mm_s, s - 1)
                buf = s % 2
                nc.scalar.copy(out=xt_sb[buf][:, 0:512], in_=xtp[buf][0][:])
                nc.scalar.copy(
                    out=xt_sb[buf][:, 512:1024], in_=xtp[buf][1][:]
                ).then_inc(xt_s, 1)

            # prologue evac: odd groups (even on DVE); slot the first two xt
            # evacs mid-sequence so mms(0) can launch right at pwe==32
            for g in range(1, nldw, 2):
                prologue_evac(scalar, g, pweB_s)
                if g == 15:
                    evac_xt(0)
                    evac_xt(1)
            # xt evac: psum fp32 -> xt_sb bf16 (cast folded into evac)
            for s in range(2, n_sys):
                evac_xt(s)

        @block.vector
        def _(vector):
            # prologue evac: psum [p, (v b)] -> V moe region [p, b(hb), o]
            # group g covers chunks c = 8g+v, all same h: o = 8*(g%16)+v
            # even groups here, odd groups on ACT
            for g in range(0, nldw, 2):
                prologue_evac(vector, g, pweA_s)
            # main: bias add evac into 1MB pair buffers (buffer 0's first use
            # produced two half-transfer incs, hence the +16 offset)
            for s in range(n_sys):
                vector.wait_ge(mm_s, s + 1)
                t = s // 2
                if t >= NB_O and s % 2 == 0:
                    vector.wait_ge(
                        dout[t % NB_O], 16 * (t // NB_O) + (16 if t % NB_O == 0 else 0)
                    )
                buf = s % 2
                half = (s % 2) * 1024
                nc.vector.tensor_add(
                    o_sb[t % NB_O][:, half : half + 512], outp[buf][0][:], bias2[:]
                )
                nc.vector.tensor_add(
                    o_sb[t % NB_O][:, half + 512 : half + 1024],
                    outp[buf][1][:],
                    bias2[:],
                ).then_inc(dve_s, 1)

    return nc


def _host_inputs(x, coeff, moe_weights, moe_bias, lin_weight, lin_bias, n_sys, core):
    """Build per-core in_map."""
    # wsb: [16v+e, 128g+i'] = W[e, o(c), 128h(c)+i'], c=8g+v, c = h*128+o
    Wr = np.asarray(moe_weights).reshape(E, 128, 2, 128)  # e,o,h,i'
    ch = Wr.transpose(2, 1, 0, 3).reshape(256, E, 128)  # c=(h,o),e,i'
    wsb = np.ascontiguousarray(
        ch.reshape(32, 8, E, 128).transpose(1, 2, 0, 3).reshape(128, 4096)
    ).astype(ml_dtypes.bfloat16)
    b0 = core * n_sys
    cT = np.asarray(coeff)[b0 : b0 + n_sys].T.astype(np.float32)  # [E, n_sys]
    c8t = np.zeros((128, 8 * n_sys), ml_dtypes.bfloat16)
    for v in range(8):
        c8t[16 * v : 16 * v + E, v * n_sys : (v + 1) * n_sys] = cT.astype(
            ml_dtypes.bfloat16
        )
    lw = np.asarray(lin_weight)  # [128, 256]
    linwt = np.ascontiguousarray(
        lw.reshape(128, 2, 128).transpose(2, 1, 0).reshape(128, 256)
    ).astype(ml_dtypes.bfloat16)
    bias_cat = np.concatenate([np.asarray(moe_bias), np.asarray(lin_bias)]).astype(
        np.float32
    )
    bias2 = np.tile(bias_cat, (128, 2))
    ident = np.eye(128, dtype=ml_dtypes.bfloat16)
    xs = np.ascontiguousarray(
        np.asarray(x)[core * n_sys * L : (core + 1) * n_sys * L]
    ).astype(ml_dtypes.bfloat16)
    return {
        "x": xs,
        "wsb": wsb,
        "c8t": c8t,
        "linwt": linwt,
        "bias2": bias2,
        "ident": ident,
    }


_CACHE = {}


def kernel(
    x,
    expert_mixing_coefficients,
    routing_idxs,
    moe_weights,
    moe_bias,
    lin_weight,
    lin_bias,
    trace=False,
    trace_cores=None,
):
    from concourse.bass_utils import run_bass_kernel_spmd

    n_sys = B // NCORES
    if "nc" not in _CACHE:
        _CACHE["nc"] = build_program(n_sys)
    nc = _CACHE["nc"]
    in_maps = [
        _host_inputs(
            x, expert_mixing_coefficients, moe_weights, moe_bias, lin_weight, lin_bias,
            n_sys, c,
        )
        for c in range(NCORES)
    ]
    res = run_bass_kernel_spmd(
        nc, in_maps, list(range(NCORES)), trace=trace, trace_cores=trace_cores
    )
    outs = [res.results[c]["out"] for c in range(NCORES)]
    full = np.concatenate(outs, axis=0)
    if trace:
        return full, res
    return full
